# revision 8
# baseline (speedup 1.0000x reference)
"""Exact self-kNN (k=32) on 8 TRN2 NeuronCores — packed-score selection.

Device (per core, SPMD over 8 cores; queries sharded, db replicated):
  Selection score S'[i,j] = x_i.x_j - |x_j|^2/2 - center_i, with
  center_i = (|x_i|^2 - 450)/2 so S' = (450 - d_ij)/2 exactly: winners
  (small d) live in [-15, 100], so 2^17*S' fits 15 bits above a 9-bit
  index field. Per 448-column chunk the PE accumulates into PSUM:
    T = (1.5*2^32 + 2^17*(-sq_j/2) + c3_i)   [bias pass, bf16 rows]
      + 2^17 * x_i.x_j (fp16 hh, two 128-halves; query scaled 2^9, db 2^8)
      - 1.5*2^32                              [bf16 const row]
      + j_local                               [fp16 iota row]
  The +B presence quantizes 2^17*S' to multiples of 512; after -B the
  value is an exact multiple of 512 and +j_local packs the column index
  into the low 9 bits losslessly. One VectorE max8 per chunk (reading
  PSUM directly) then yields value+index together — no max_index pass,
  no gather. A 5-round max8/max_index/match_replace merge over the 296
  per-chunk candidates gives the top-40 packed values + chunk positions;
  tiny int ops decode global indices. Device outputs top-40 indices.

Host: exact fp32 distances for the 40 candidates per query (same
recompute as the reference), stable (d, idx) sort, keep 32. This fixes
quantization-order flips; accuracy matches the fp32-exact baseline
(44 mismatched tie elements of 524288, dist rel err ~1e-6).
"""

import numpy as np

N = 16384
D = 256
K = 32
KDEV = 40                     # device returns top-40 candidates per row
NCORES = 8
QPC = N // NCORES             # 2048 queries per core
QTILES = QPC // 128           # 16
CHUNK = 448
NCH = 37                      # 36*448 + 256
CHUNKS = [CHUNK] * 36 + [256]
NCAND = NCH * 8               # 296
B_CONST = 1.5 * 2.0**32       # exact in bf16
SC_Q = 512.0                  # query operand scale (2^9)
SC_D = 256.0                  # db operand scale (2^8)

_nc_cache = None


def _build():
    import concourse.bacc as bacc
    import concourse.mybir as mybir
    import concourse.tile as tile

    nc = bacc.Bacc(trn_type="TRN2")
    f32, f16 = mybir.dt.float32, mybir.dt.float16
    bf16 = mybir.dt.bfloat16
    i32, u16 = mybir.dt.int32, mybir.dt.uint16
    Alu = mybir.AluOpType

    hq0_in = nc.dram_tensor("hq0", [128, QPC], f16, kind="ExternalInput")
    hq1_in = nc.dram_tensor("hq1", [128, QPC], f16, kind="ExternalInput")
    hT0_in = nc.dram_tensor("hT0", [128, N], f16, kind="ExternalInput")
    hT1_in = nc.dram_tensor("hT1", [128, N], f16, kind="ExternalInput")
    brow_in = nc.dram_tensor("brow", [5, N], bf16, kind="ExternalInput")
    bstat_in = nc.dram_tensor("bstat", [5, QPC], bf16, kind="ExternalInput")
    iota_in = nc.dram_tensor("iotarow", [1, N], f16, kind="ExternalInput")
    out_i = nc.dram_tensor("out_i", [QPC, KDEV], i32, kind="ExternalOutput")

    with tile.TileContext(nc) as tc:
        with (
            tc.tile_pool(name="db", bufs=1) as db,
            tc.tile_pool(name="work", bufs=3) as work,
            tc.tile_pool(name="scp", bufs=12) as scp,
            tc.tile_pool(name="ps", bufs=7, space="PSUM") as ps,
        ):
            # ---------------- resident inputs ----------------
            hq = [db.tile([128, QPC], f16, name=f"hq{i}") for i in range(2)]
            nc.sync.dma_start(hq[0][:], hq0_in[:, :])
            nc.sync.dma_start(hq[1][:], hq1_in[:, :])
            hT = [db.tile([128, N], f16, name=f"hT{i}") for i in range(2)]
            SL = 2048
            for half, src in ((0, hT0_in), (1, hT1_in)):
                for s0 in range(0, N, SL):
                    sl = slice(s0, s0 + SL)
                    nc.sync.dma_start(hT[half][:, sl], src[:, sl])
            brow_sb = db.tile([5, N], bf16, name="brow")
            nc.sync.dma_start(brow_sb[:], brow_in[:, :])
            bstat_sb = db.tile([5, QPC], bf16, name="bstat")
            nc.sync.dma_start(bstat_sb[:], bstat_in[:, :])
            iota_sb = db.tile([1, N], f16, name="iotarow")
            nc.sync.dma_start(iota_sb[:], iota_in[:, :])

            # ---------------- constants ----------------
            negB = db.tile([1, CHUNK], bf16)
            nc.vector.memset(negB[:], -B_CONST)
            ones_bf = db.tile([1, 128], bf16)
            nc.vector.memset(ones_bf[:], 1.0)
            ones_16 = db.tile([1, 128], f16)
            nc.vector.memset(ones_16[:], 1.0)
            c511 = db.tile([128, 1], i32)
            nc.vector.memset(c511[:], 511)
            cfff8 = db.tile([128, 1], i32)
            nc.vector.memset(cfff8[:], 65528)      # 0xFFF8
            c56 = db.tile([128, 1], i32)
            nc.vector.memset(c56[:], 56)           # 448/8
            zero_i = db.tile([128, 1], i32)
            nc.vector.memset(zero_i[:], 0)

            # ---------------- main loop over query tiles ----------------
            for t in range(QTILES):
                qs = slice(128 * t, 128 * (t + 1))
                v_cand = work.tile([128, NCAND], f32, tag="v_cand")
                import contextlib
                sc = (lambda nm: nc.named_scope(nm)) if t == 8 else (
                    lambda nm: contextlib.nullcontext())
                # pass-major over groups of 7 chunks: each of the 5 passes
                # sweeps the whole group with one stationary load, so the PE
                # streams back-to-back matmuls and stays at 2.4 GHz.
                GRP = 7
                with sc("chunkstage"):
                 for g0 in range(0, NCH, GRP):
                    cl = list(range(g0, min(NCH, g0 + GRP)))
                    psums = [ps.tile([128, CHUNKS[c]], f32, tag="psum",
                                     name="psum")
                             for c in cl]
                    def _cs(c):
                        return slice(CHUNK * c, CHUNK * c + CHUNKS[c])
                    for i, c in enumerate(cl):
                        nc.tensor.matmul(psums[i][:], bstat_sb[:, qs],
                                         brow_sb[:, _cs(c)],
                                         start=True, stop=False)
                    for i, c in enumerate(cl):
                        nc.tensor.matmul(psums[i][:], hq[0][:, qs],
                                         hT[0][:, _cs(c)],
                                         start=False, stop=False)
                    for i, c in enumerate(cl):
                        nc.tensor.matmul(psums[i][:], hq[1][:, qs],
                                         hT[1][:, _cs(c)],
                                         start=False, stop=False)
                    for i, c in enumerate(cl):
                        nc.tensor.matmul(psums[i][:], ones_bf[:],
                                         negB[:, :CHUNKS[c]],
                                         start=False, stop=False)
                    for i, c in enumerate(cl):
                        nc.tensor.matmul(psums[i][:], ones_16[:],
                                         iota_sb[:, _cs(c)],
                                         start=False, stop=True)
                    # ScalarE stages PSUM->SBUF so the PE's bank-free waits
                    # are always satisfied (deep SBUF pool decouples DVE lag)
                    for i, c in enumerate(cl):
                        s_sb = scp.tile([128, CHUNKS[c]], f32, tag="s_sb",
                                        name="s_sb")
                        nc.scalar.copy(s_sb[:], psums[i][:])
                        nc.vector.max(out=v_cand[:, 8 * c:8 * c + 8],
                                      in_=s_sb[:])

                # merge: global top-40 of the candidate table
                with sc("merge"):
                    v_work = work.tile([128, NCAND], f32, tag="v_work")
                    nc.scalar.copy(v_work[:], v_cand[:])
                    v40 = work.tile([128, KDEV], f32, tag="v40")
                    p_u = work.tile([128, KDEV], u16, tag="p_u")
                    for r in range(KDEV // 8):
                        nc.vector.max(out=v40[:, 8 * r:8 * r + 8], in_=v_work[:])
                        nc.vector.max_index(
                            out=p_u[:, 8 * r:8 * r + 8],
                            in_max=v40[:, 8 * r:8 * r + 8],
                            in_values=v_work[:],
                        )
                        if r < KDEV // 8 - 1:
                            nc.vector.match_replace(
                                out=v_work[:], in_to_replace=v40[:, 8 * r:8 * r + 8],
                                in_values=v_work[:], imm_value=-3e38,
                            )

                # decode: global index = (p_u >> 3)*448 + (T2 mod 512)
                with sc("decode"):
                    t32 = work.tile([128, KDEV], i32, tag="t32")
                    nc.vector.tensor_copy(t32[:], v40[:])
                    j32 = work.tile([128, KDEV], i32, tag="j32")
                    nc.vector.scalar_tensor_tensor(
                        out=j32[:], in0=t32[:], scalar=c511[:, 0:1],
                        in1=zero_i[:, 0:1].to_broadcast([128, KDEV]),
                        op0=Alu.bitwise_and, op1=Alu.bitwise_or,
                    )
                    pu32 = work.tile([128, KDEV], i32, tag="pu32")
                    nc.vector.tensor_copy(pu32[:], p_u[:])
                    m1 = work.tile([128, KDEV], i32, tag="m1")
                    nc.vector.scalar_tensor_tensor(
                        out=m1[:], in0=pu32[:], scalar=cfff8[:, 0:1],
                        in1=zero_i[:, 0:1].to_broadcast([128, KDEV]),
                        op0=Alu.bitwise_and, op1=Alu.bitwise_or,
                    )
                    gi = work.tile([128, KDEV], i32, tag="gi")
                    nc.vector.scalar_tensor_tensor(
                        out=gi[:], in0=m1[:], scalar=c56[:, 0:1],
                        in1=j32[:], op0=Alu.mult, op1=Alu.add,
                    )
                    # slot 0 is always the self-match: overwrite with row id
                    nc.gpsimd.iota(gi[:, 0:1], pattern=[[1, 1]], base=128 * t,
                                   channel_multiplier=1)

                nc.sync.dma_start(out_i[qs, :], gi[:])
    nc.finalize()
    return nc


def make_in_maps(x):
    """Host-side prep: fp16/bf16 operand splits + bias tables per core."""
    import ml_dtypes

    x = np.ascontiguousarray(np.asarray(x, dtype=np.float32))
    xT = x.T  # [256, N]
    h9 = (xT * np.float32(SC_Q)).astype(np.float16)   # query-side, scale 2^9
    h8 = (xT * np.float32(SC_D)).astype(np.float16)   # db-side, scale 2^8
    sq32 = ((x.astype(np.float64) ** 2).sum(1)).astype(np.float32)
    bias_v = (np.float32(-(2.0**16)) * sq32).astype(np.float32)  # 2^17*(-sq/2)
    b0 = bias_v.astype(ml_dtypes.bfloat16)
    r = (bias_v - b0.astype(np.float32)).astype(np.float32)
    b1 = r.astype(ml_dtypes.bfloat16)
    b2 = (r - b1.astype(np.float32)).astype(ml_dtypes.bfloat16)
    ones_n = np.ones(N, dtype=ml_dtypes.bfloat16)
    bB = np.full(N, B_CONST, dtype=ml_dtypes.bfloat16)
    brow = np.ascontiguousarray(
        np.stack([b0, b1, b2, ones_n, bB]))            # [5, N] bf16
    c3 = (np.float32(-(2.0**16)) * (sq32 - np.float32(450.0))
          ).astype(ml_dtypes.bfloat16)                 # per-query center row
    iota_row = np.ascontiguousarray(
        (np.arange(N, dtype=np.int64) % CHUNK).astype(np.float16)[None, :])

    in_maps = []
    for core in range(NCORES):
        qs = slice(core * QPC, (core + 1) * QPC)
        ones_q = np.ones(QPC, dtype=ml_dtypes.bfloat16)
        bstat = np.ascontiguousarray(
            np.stack([ones_q, ones_q, ones_q, c3[qs], ones_q]))  # [5, QPC]
        in_maps.append({
            "hq0": np.ascontiguousarray(h9[:128, qs]),
            "hq1": np.ascontiguousarray(h9[128:, qs]),
            "hT0": np.ascontiguousarray(h8[:128]),
            "hT1": np.ascontiguousarray(h8[128:]),
            "brow": brow,
            "bstat": bstat,
            "iotarow": iota_row,
        })
    return in_maps


def kernel(x, k):
    from concourse.bass_utils import run_bass_kernel_spmd

    global _nc_cache
    x = np.ascontiguousarray(np.asarray(x, dtype=np.float32))
    assert x.shape == (N, D)
    assert int(k) == K

    if _nc_cache is None:
        _nc_cache = _build()
    nc = _nc_cache

    in_maps = make_in_maps(x)
    res = run_bass_kernel_spmd(nc, in_maps, core_ids=list(range(NCORES)))
    idx40 = np.concatenate([r["out_i"] for r in res.results], axis=0)
    idx40 = idx40.astype(np.int64)  # [N, KDEV]
    # slot 0 is always the self-match; the device writes core-local row ids,
    # so restore the global ids here.
    idx40[:, 0] = np.arange(N)

    # host refine: exact fp32 distances for the 40 candidates, sort, keep 32
    d40 = np.empty((N, KDEV), np.float32)
    for r0 in range(0, N, 1024):
        blk = slice(r0, min(N, r0 + 1024))
        diff = x[blk][:, None, :] - x[idx40[blk]]
        d40[blk] = (diff * diff).sum(-1)
    sidx = np.lexsort((idx40, d40), axis=1)[:, :K]
    idx = np.take_along_axis(idx40, sidx, axis=1).astype(np.int32)
    dist = np.take_along_axis(d40, sidx, axis=1).astype(np.float32)
    return idx, dist
